# revision 45
# baseline (speedup 1.0000x reference)
"""Trainium2 Bass kernel for nn_Attention_9242769622327.

Math: the reference computes
    qkv = x @ W1.T ; q,k,v = split(qkv)
    score = softmax(k^T v / 4, axis=-1)            # rows sum to 1
    attn  = softmax(einsum('bhnk,bhkc->bhnk', q/4, score), axis=-1)
          = softmax(q/4)                           # k/v are mathematically dead
    out   = attn @ W2.T
so only the q-projection (first E rows of W1), a per-head (64-wide) softmax,
and the output projection are needed.

Distribution: pure data-parallel over the 32768 = B*S rows; each of the 8
cores handles 4096 rows with the full (transposed) weights. No collectives.

EVERY matmul runs as fp8 e4m3 MatmulPerfMode.DoubleRow (two K-tiles per
instruction, 2x PE throughput; measured 216ns per [*,2,*]x[*,2,512]
instruction back-to-back — and mixing any f16 matmul into the fp8 stream
was measured to stall the PE at ~2x cycle time for ~us, so the PE stream
is kept mode-pure).  fp8's ~2.7% per-value quantization noise is tamed by
centering every fp8-quantized quantity around its known mean:
  - mm1 (q-projection): softmax's /4 temperature + normalization damp the
    error ~4x; plain fp8 x / fp8 (32*W1q) gives ~0.9% final.
  - attn: rows sum to exactly 1 per 64-wide head, so the PE gets
    at0 = (u - 1)*rb   (u = exp(q/4) f16, rb = A*64/s broadcast, A=16)
    which is ~4x smaller than u*rb.
  - rcp: rcpc = A*64/s - A (+-2.5) is quantized fp8; the exact constant A
    is restored through a second K-tile whose selector row multiplies a
    constant-A row, so rb = selt0^T@rcpc + A exactly in fp32 PSUM.
  - mm2 constant part: sum_h rcp*w2sum splits into the exact f32 bias
    colsum(32*W2T)/2048 (applied in the output-copy ACT) plus the tiny
    centered fp8 matmul rcpc @ w2sum8.
  - head sums: s comes from an fp8 copy of u (DVE); the 2.7%/sqrt(64)
    coherent error this adds is ~0.3%.
Host-emulated + CoreSim rel err: 1.35e-2 (gate 2e-2).

On-chip layout is fully transposed (features on partitions, rows on the
free dim) so no on-chip transposes are needed anywhere:
    qT[n,m]  = sum_k W1qT[k,n]*xT[k,m]        (PE DR, fp8, PSUM=32q)
    u        = exp(qT/128)                    (ACT, PSUM->SBUF f16)
    u8       = fp8(u)                         (DVE copy, pair tiles)
    s[g,m]/64= sum_{n in g} u8[n,m]/64        (PE DR w/ 1/64 selector pairs)
    rcpc     = fp8(A*64/s - A)                (DVE reciprocal + tensor_scalar)
    rb[n,m]  = rcpc[head(n),m] + A            (PE DR w/ selector^T + const row)
    at0      = (u - 1)*rb                     (DVE scalar_tensor_tensor, fp8)
    outT[j,m]= [rcpc@w2sum8 + sum_n W2T[n,j]*at0[n,m]]/(A*64*32) + bias_j
                                              (PE DR; ACT Identity w/ bias)

Per stripe the PE issues 84 uniform DR matmuls: 8 rb (interleaved into the
mm1 loop so the PE stays ahead of the DVE's at0 chain), 32 mm1, 40 mm2
(8 j-groups of [rcpc-corr + 4 at0]), 4 sel — ~218ns each sustained.
"""

import sys

sys.path.insert(0, "/opt/trn_rl_repo")

import numpy as np
import ml_dtypes

import concourse.bass as bass
import concourse.bacc as bacc
import concourse.tile as tile
from concourse import mybir
from concourse.bass_utils import run_bass_kernel_spmd

F16 = mybir.dt.float16
FP8 = mybir.dt.float8e4
F32 = mybir.dt.float32
AF = mybir.ActivationFunctionType
ALU = mybir.AluOpType
DR = mybir.MatmulPerfMode.DoubleRow

N_CORES = 8
B, S, E = 4, 8192, 1024
HEADS, HEAD_DIM = 16, 64
M_TOTAL = B * S                # 32768
M_CORE = M_TOTAL // N_CORES    # 4096 rows per core
MS = 512                       # m-stripe width (moving free dim / PSUM bank)
KC2 = E // 256                 # 4 DoubleRow contraction pairs
NC_ = E // 128                 # 8 feature chunks
A_SCALE = 16.0                 # fp8 scale for the centered attn
W_SCALE = 32.0                 # host pre-scale of W1q/W2 (std 1/32 -> ~1)
OUT_DESCALE = 1.0 / (A_SCALE * 64.0 * W_SCALE)

_E4 = ml_dtypes.float8_e4m3
_F16 = np.float16


def build_nc(m_core=M_CORE) -> bass.Bass:
    n_stripes = m_core // MS
    nc = bacc.Bacc("TRN2", debug=False)

    xt = nc.dram_tensor("xt", [E, m_core], FP8, kind="ExternalInput")
    w1t = nc.dram_tensor("w1t", [E, E], FP8, kind="ExternalInput")
    w2t = nc.dram_tensor("w2t", [E, E], FP8, kind="ExternalInput")
    sel8 = nc.dram_tensor("sel8", [128, KC2 * 2 * 128], FP8, kind="ExternalInput")
    biasd = nc.dram_tensor("biasd", [128, NC_], F32, kind="ExternalInput")
    # DRAM staging for the per-head reciprocal broadcast: SBUF->SBUF DMAs
    # reject zero-stride partition steps, but a DRAM source is a flat
    # address pattern, so the 64x per-head replication reads DRAM with a
    # stride-0 dim instead of 8 PE broadcast matmuls per stripe.
    rbstage = nc.dram_tensor("rbstage", [2, HEADS, MS], F16, kind="Internal")
    outT = nc.dram_tensor("outT", [E, m_core], F16, kind="ExternalOutput")

    xt_v = xt[:, :].rearrange("(c p) m -> p c m", p=128)    # [128, 8, m_core]
    w1_v = w1t[:, :].rearrange("(c p) n -> p c n", p=128)   # [128, 8, 1024]
    w2_v = w2t[:, :].rearrange("(c p) j -> p c j", p=128)   # [128, 8, 1024]

    with tile.TileContext(nc) as tc:
        with (
            tc.tile_pool(name="weights", bufs=1) as wpool,
            tc.tile_pool(name="xt", bufs=n_stripes) as xpool,
            tc.tile_pool(name="u", bufs=16) as upool,
            tc.tile_pool(name="u8", bufs=4) as u8pool,
            tc.tile_pool(name="tmul", bufs=3) as tpool,
            tc.tile_pool(name="rb", bufs=2) as rbpool,
            tc.tile_pool(name="at", bufs=8) as apool,
            tc.tile_pool(name="small", bufs=2) as spool,
            tc.tile_pool(name="ostage", bufs=8) as opool,
            tc.tile_pool(name="ps_q", bufs=2, space="PSUM") as psq,
            tc.tile_pool(name="ps_s", bufs=1, space="PSUM") as pss,
            tc.tile_pool(name="ps_rb", bufs=2, space="PSUM") as psrb,
            tc.tile_pool(name="ps_o", bufs=3, space="PSUM") as pso,
        ):
            # Warm the PE's HAM clock gate with small throwaway DR matmuls
            # (same mode as the real stream) while the first DMAs are in
            # flight.  gpsimd memset is ready right after its preamble
            # (~6.5us), before the PE's own preamble ends; N=128 keeps each
            # cold-clock warm matmul cheap so they don't delay stripe 0.
            warm_sb = wpool.tile([128, 2, 128], FP8, name="warm_sb")
            nc.gpsimd.memset(warm_sb[:], 0.0)
            warm_ps = psq.tile([128, MS], F32, tag="q", name="warm_ps")
            for _ in range(14):
                nc.tensor.matmul(
                    warm_ps[:, 0:128], warm_sb[:], warm_sb[:],
                    start=True, stop=True, perf_mode=DR,
                )

            # Weight/selector loads, ordered so stripe 0's mm1 can start
            # as early as possible.
            w1_k = []   # 4 tiles [128, 2, E] fp8 (DoubleRow pairs)
            x_k = [[None] * KC2 for _ in range(n_stripes)]
            # w1 on the sync HWDGE queue, stripe-0 x on the scalar-engine
            # HWDGE queue: both transfer in parallel so stripe 0's first
            # matmul isn't gated by a serial queue (ACT is idle here).
            # Balance stripe 0's 1.5MB of first-needed tiles across the two
            # fast HWDGE issue channels (~768KB each) so mm1 starts earlier
            # (gpsimd's SWDGE path measured too slow to carry weight tiles).
            w1_eng = [nc.sync, nc.sync, nc.scalar, nc.sync]
            x0_eng = [nc.scalar, nc.scalar, nc.scalar, nc.scalar]
            for kc2 in range(KC2):
                t = wpool.tile([128, 2, E], FP8, name=f"w1k{kc2}")
                w1_eng[kc2].dma_start(t[:], w1_v[:, 2 * kc2:2 * kc2 + 2, :])
                w1_k.append(t)
                tx = xpool.tile([128, 2, MS], FP8, tag=f"xt_{kc2}", name=f"xt0_{kc2}")
                x0_eng[kc2].dma_start(tx[:], xt_v[:, 2 * kc2:2 * kc2 + 2, 0:MS])
                x_k[0][kc2] = tx
            sel8_t = wpool.tile([128, KC2, 2, 128], FP8, name="sel8_t")
            nc.sync.dma_start(
                sel8_t[:], sel8[:, :].rearrange("p (c i g) -> p c i g", i=2, g=128)
            )
            bias_t = wpool.tile([128, NC_], F32, name="bias_t")
            nc.sync.dma_start(bias_t[:], biasd[:, :])

            w2_k = []   # 4 tiles [128, 2, E] fp8 (pairs of n-chunks)
            for c2 in range(KC2):
                t = wpool.tile([128, 2, E], FP8, name=f"w2k{c2}")
                nc.sync.dma_start(t[:], w2_v[:, 2 * c2:2 * c2 + 2, :])
                w2_k.append(t)

            def prefetch_x(ms):
                if ms < 1 or ms >= n_stripes:
                    return
                for kc2 in range(KC2):
                    tx = xpool.tile(
                        [128, 2, MS], FP8, tag=f"xt_{kc2}", name=f"xt{ms}_{kc2}"
                    )
                    nc.sync.dma_start(
                        tx[:], xt_v[:, 2 * kc2:2 * kc2 + 2, ms * MS:(ms + 1) * MS]
                    )
                    x_k[ms][kc2] = tx
            prefetch_x(1)

            def emit_rb_at0(prev, ci):
                """The u*rb - A DVE chain -> fp8 (rb comes from the DMA
                broadcast tile).  at0 = u*rb - A is centered (the exact
                constant A*colsum(W2) flows through the f32 bias in the
                output copy), so mm2 needs no per-head correction."""
                pair, half = divmod(ci, 2)
                t_f = tpool.tile([128, MS], F16, tag="t", name="t_f")
                nc.vector.tensor_mul(
                    t_f[:], prev["u"][ci][:], prev["rb"][:, ci, :]
                )
                nc.vector.tensor_scalar(
                    prev["at"][pair][:, half, :], t_f[:], -A_SCALE, None,
                    op0=ALU.add,
                )

            def emit_mm2(prev, js):
                """j-groups: 4 fp8 DR at0 matmuls each."""
                for j in js:
                    o_ps = pso.tile([128, MS], F32, tag="o", name="o_ps")
                    for c2 in range(KC2):
                        nc.tensor.matmul(
                            o_ps[:],
                            w2_k[c2][:, :, j * 128:(j + 1) * 128],
                            prev["at"][c2][:],
                            start=(c2 == 0),
                            stop=(c2 == KC2 - 1),
                            perf_mode=DR,
                            skip_group_check=True,
                        )
                    o_t = opool.tile([128, MS], F16, tag="ost", name="o_t")
                    nc.scalar.activation(
                        o_t[:], o_ps[:], AF.Identity,
                        bias=bias_t[:, j:j + 1], scale=OUT_DESCALE,
                    )
                    nc.sync.dma_start(
                        outT[j * 128:(j + 1) * 128,
                             prev["ms"] * MS:(prev["ms"] + 1) * MS],
                        o_t[:],
                    )

            def emit_sel(cur):
                """4 DR head-sum matmuls on the fp8 u pairs + rcp chain."""
                s_ps = pss.tile([128, MS], F32, tag="s", name="s_ps")
                for c2 in range(KC2):
                    nc.tensor.matmul(
                        s_ps[:],
                        sel8_t[:, c2, :, :],
                        cur["u8"][c2][:],
                        start=(c2 == 0),
                        stop=(c2 == KC2 - 1),
                        perf_mode=DR,
                    )
                rcp32 = spool.tile([HEADS, MS], F32, tag="rcp32", name="rcp32")
                nc.vector.reciprocal_approx_fast(rcp32[:], s_ps[0:HEADS, :])
                rcp16 = spool.tile([HEADS, MS], F16, tag="rcp16", name="rcp16")
                nc.vector.tensor_scalar(
                    rcp16[:], rcp32[:], A_SCALE, None, op0=ALU.mult
                )
                # stage to DRAM, then broadcast each head row to its 64
                # partitions; same sync queue => write-before-read order.
                par = cur["ms"] % 2
                nc.sync.dma_start(rbstage[par, :, :], rcp16[:])
                for g in range(2):
                    src = rbstage[par, :, :].rearrange(
                        "(c g) m -> g c m", g=2
                    )[g].rearrange("c (o m) -> o c m", o=1).broadcast_to(
                        [64, NC_, MS]
                    )
                    nc.sync.dma_start(cur["rb"][64 * g:64 * g + 64, :, :], src)

            # Two-deep software pipeline: block i runs mm1(i), the rb/at0
            # chain of stripe i-1, and mm2 of stripe i-2 — so the DVE's
            # at0 chain always has a full block of slack before mm2
            # consumes it, and the tail drain runs fully fed.
            prev = None
            prev2 = None
            for ms in range(n_stripes):
                prefetch_x(ms + 1)
                cur = {
                    "ms": ms,
                    "u": [],
                    "u8": [
                        u8pool.tile([128, 2, MS], FP8, tag=f"u8{p}", name=f"u8{ms}_{p}")
                        for p in range(KC2)
                    ],
                    "at": [
                        apool.tile([128, 2, MS], FP8, tag=f"at{p}", name=f"at{ms}_{p}")
                        for p in range(KC2)
                    ],
                    "rb": rbpool.tile(
                        [128, NC_, MS], F16, tag="rb", name=f"rb{ms}"
                    ),
                }

                # Head of block: 2 rb matmuls of the previous stripe; the
                # rest interleave into the mm1 loop so the PE stays just
                # ahead of the DVE's 823ns/op at0 chain (psrb has 2 bufs).
                if prev is not None:
                    emit_rb_at0(prev, 0)
                    emit_rb_at0(prev, 1)

                for ci in range(NC_):
                    q_ps = psq.tile([128, MS], F32, tag="q", name="q_ps")
                    for kc2 in range(KC2):
                        nc.tensor.matmul(
                            q_ps[:],
                            w1_k[kc2][:, :, ci * 128:(ci + 1) * 128],
                            x_k[ms][kc2][:],
                            start=(kc2 == 0),
                            stop=(kc2 == KC2 - 1),
                            perf_mode=DR,
                        )
                    u_t = upool.tile([128, MS], F16, tag="u", name="u_t")
                    nc.scalar.activation(u_t[:], q_ps[:], AF.Exp, scale=1.0 / 128.0)
                    cur["u"].append(u_t)
                    if prev is not None and ci + 2 < NC_:
                        emit_rb_at0(prev, ci + 2)

                # u8 copies (DVE) emitted after all at0 ops so the at0
                # chain (needed first, by mm2) drains first.
                for ci in range(NC_):
                    nc.vector.tensor_copy(
                        cur["u8"][ci // 2][:, ci % 2, :], cur["u"][ci][:]
                    )

                if prev2 is not None:
                    emit_mm2(prev2, range(6))
                emit_sel(cur)
                if prev2 is not None:
                    emit_mm2(prev2, range(6, NC_))
                prev2, prev = prev, cur

            # drain block: last stripe's rb/at0 chain interleaved with the
            # second-to-last stripe's output projection
            emit_rb_at0(prev, 0)
            emit_rb_at0(prev, 1)
            for k in range(6):
                if prev2 is not None:
                    emit_mm2(prev2, [k])
                emit_rb_at0(prev, k + 2)
            if prev2 is not None:
                emit_mm2(prev2, range(6, NC_))
            # epilogue: last stripe's output projection, fully fed
            emit_mm2(prev, range(NC_))
    nc.compile()
    return nc


_NC_CACHE = None
LAST_RESULT = None


def _ensure_ntff_hook():
    """bass_utils' axon trace path needs antenv.axon_hooks, which this
    container's antenv lacks. Provide it + register the ctypes NTFF hook."""
    import types

    try:
        from antenv.axon_hooks import get_axon_ntff_profile_hook  # noqa: F401
        return True
    except ImportError:
        pass
    try:
        import antenv
        from trn_agent_boot.trn_boot import _ntff_profile_via_ctypes

        m = types.ModuleType("antenv.axon_hooks")
        state = {"hook": None}
        m.set_axon_ntff_profile_hook = lambda h: state.__setitem__("hook", h)
        m.get_axon_ntff_profile_hook = lambda: state["hook"]
        sys.modules["antenv.axon_hooks"] = m
        antenv.axon_hooks = m
        m.set_axon_ntff_profile_hook(
            _ntff_profile_via_ctypes("/opt/axon/libaxon_pjrt.so")
        )
        return True
    except Exception as e:  # pragma: no cover
        print(f"ntff hook injection failed: {e}")
        return False


def _selectors():
    # head index of global feature n is n // 64; chunk ci covers n in
    # [128ci, 128ci+128) -> heads 2ci (partitions 0..63), 2ci+1 (64..127)
    # All selector tiles are padded to the uniform (128, 2, 128) shape.
    sel8 = np.zeros((128, KC2, 2, 128), np.float32)
    for c2 in range(KC2):
        for i in range(2):
            ci = 2 * c2 + i
            sel8[:64, c2, i, 2 * ci] = 1.0 / 64.0
            sel8[64:, c2, i, 2 * ci + 1] = 1.0 / 64.0
    return np.ascontiguousarray(sel8.reshape(128, KC2 * 2 * 128)).astype(_E4)


def _prep_weights(W1, W2):
    w1t = np.ascontiguousarray(W1[:E, :].T * W_SCALE).astype(_E4)  # [k, n]
    w2t = np.ascontiguousarray(W2.T * W_SCALE).astype(_E4)         # [n, j]
    # exact column sums of 32*W2T (fp32, TRUE weights — this is what keeps
    # the W2 fp8 error coupled only to the centered at0)
    colsum = (W2.T * W_SCALE).sum(axis=0)
    bias = (colsum * A_SCALE * OUT_DESCALE).astype(np.float32)
    biasd = np.ascontiguousarray(bias.reshape(NC_, 128).T)         # [128, NC_]
    return w1t, w2t, biasd


def kernel(x, W1, W2, heads, trace=False):
    global _NC_CACHE, LAST_RESULT
    x = np.asarray(x, dtype=np.float32)
    W1 = np.asarray(W1, dtype=np.float32)
    W2 = np.asarray(W2, dtype=np.float32)

    X = x.reshape(M_TOTAL, E)
    XT8 = np.ascontiguousarray(X.T).astype(_E4)  # [E, M_TOTAL]
    w1t, w2t, biasd = _prep_weights(W1, W2)
    sel8 = _selectors()

    in_maps = []
    for c in range(N_CORES):
        xt_c = np.ascontiguousarray(XT8[:, c * M_CORE:(c + 1) * M_CORE])
        in_maps.append(
            {"xt": xt_c, "w1t": w1t, "w2t": w2t, "sel8": sel8,
             "biasd": biasd}
        )

    if _NC_CACHE is None:
        _NC_CACHE = build_nc()

    if trace:
        trace = _ensure_ntff_hook()

    res = run_bass_kernel_spmd(_NC_CACHE, in_maps, list(range(N_CORES)), trace=trace)
    LAST_RESULT = res

    OT = np.concatenate(
        [np.asarray(res.results[c]["outT"]).astype(np.float32) for c in range(N_CORES)],
        axis=1,
    )
    return np.ascontiguousarray(OT.T).reshape(B, S, E)


# revision 47
# speedup vs baseline: 1.0439x; 1.0439x over previous
"""Trainium2 Bass kernel for nn_Attention_9242769622327.

Math: the reference computes
    qkv = x @ W1.T ; q,k,v = split(qkv)
    score = softmax(k^T v / 4, axis=-1)            # rows sum to 1
    attn  = softmax(einsum('bhnk,bhkc->bhnk', q/4, score), axis=-1)
          = softmax(q/4)                           # k/v are mathematically dead
    out   = attn @ W2.T
so only the q-projection (first E rows of W1), a per-head (64-wide) softmax,
and the output projection are needed.

Distribution: pure data-parallel over the 32768 = B*S rows; each of the 8
cores handles 4096 rows with the full (transposed) weights. No collectives.

EVERY matmul runs as fp8 e4m3 MatmulPerfMode.DoubleRow (two K-tiles per
instruction, 2x PE throughput; measured 216ns per [*,2,*]x[*,2,512]
instruction back-to-back — and mixing any f16 matmul into the fp8 stream
was measured to stall the PE at ~2x cycle time for ~us, so the PE stream
is kept mode-pure).  fp8's ~2.7% per-value quantization noise is tamed by
centering every fp8-quantized quantity around its known mean:
  - mm1 (q-projection): softmax's /4 temperature + normalization damp the
    error ~4x; plain fp8 x / fp8 (32*W1q) gives ~0.9% final.
  - attn: rows sum to exactly 1 per 64-wide head, so the PE gets
    at0 = u*rb - A   (u = exp(q/4) f16, rb = A*64/s broadcast, A=16)
    which is ~4x smaller than u*rb; the exact complement A*colsum(32*W2T)
    /2048 is a per-partition f32 bias in the output-copy ACT.  The bias
    MUST use the true (unquantized) colsums — that is what keeps W2's fp8
    error coupled only to the centered at0 (quantized colsums measured
    2.9% total err vs 1.34%).
  - rcp: rcpc = A*64/s - A (+-2.5) is quantized fp8; the exact constant A
    is restored through a second K-tile whose selector row multiplies a
    constant-A row, so rb = selt0^T@rcpc + A exactly in fp32 PSUM.
  - head sums: s comes from an fp8 copy of u (DVE); the 2.7%/sqrt(64)
    coherent error this adds is ~0.3%.
Host-emulated + CoreSim + hardware rel err: 1.342e-2 (gate 2e-2).

On-chip layout is fully transposed (features on partitions, rows on the
free dim) so no on-chip transposes are needed anywhere:
    qT[n,m]  = sum_k W1qT[k,n]*xT[k,m]        (PE DR, fp8, PSUM=32q)
    u        = exp(qT/128)                    (ACT, PSUM->SBUF f16)
    u8       = fp8(u)                         (DVE copy, pair tiles)
    s[g,m]/64= sum_{n in g} u8[n,m]/64        (PE DR w/ 1/64 selector pairs)
    rcpc     = fp8(A*64/s - A)                (DVE reciprocal + tensor_scalar)
    rb[n,m]  = rcpc[head(n),m] + A            (PE DR w/ selector^T + const row)
    at0      = u*rb - A                       (DVE tensor_mul + tensor_scalar,
                                               fp8)
    outT[j,m]= [sum_n W2T[n,j]*at0[n,m]]/(A*64*32) + bias_j
                                              (PE DR; ACT Identity w/ bias)

Per stripe the PE issues 76 uniform DR matmuls: 8 rb (interleaved into the
mm1 loop so the PE stays ahead of the DVE's at0 chain), 32 mm1, 32 mm2
(8 j-groups of 4), 4 sel — ~216ns each sustained.  A two-deep software
pipeline (block i: mm1(i), rb/at0(i-1), mm2(i-2)) gives the DVE chain a
full block of slack and lets the tail drain run fully fed.
"""

import sys

sys.path.insert(0, "/opt/trn_rl_repo")

import numpy as np
import ml_dtypes

import concourse.bass as bass
import concourse.bacc as bacc
import concourse.tile as tile
from concourse import mybir
from concourse.bass_utils import run_bass_kernel_spmd

F16 = mybir.dt.float16
FP8 = mybir.dt.float8e4
F32 = mybir.dt.float32
AF = mybir.ActivationFunctionType
ALU = mybir.AluOpType
DR = mybir.MatmulPerfMode.DoubleRow

N_CORES = 8
B, S, E = 4, 8192, 1024
HEADS, HEAD_DIM = 16, 64
M_TOTAL = B * S                # 32768
M_CORE = M_TOTAL // N_CORES    # 4096 rows per core
MS = 512                       # m-stripe width (moving free dim / PSUM bank)
KC2 = E // 256                 # 4 DoubleRow contraction pairs
NC_ = E // 128                 # 8 feature chunks
A_SCALE = 16.0                 # fp8 scale for the centered attn
W_SCALE = 32.0                 # host pre-scale of W1q/W2 (std 1/32 -> ~1)
OUT_DESCALE = 1.0 / (A_SCALE * 64.0 * W_SCALE)

_E4 = ml_dtypes.float8_e4m3
_F16 = np.float16


def build_nc(m_core=M_CORE) -> bass.Bass:
    n_stripes = m_core // MS
    nc = bacc.Bacc("TRN2", debug=False)

    xt = nc.dram_tensor("xt", [E, m_core], FP8, kind="ExternalInput")
    w1t = nc.dram_tensor("w1t", [E, E], FP8, kind="ExternalInput")
    w2t = nc.dram_tensor("w2t", [E, E], FP8, kind="ExternalInput")
    sel8 = nc.dram_tensor("sel8", [128, KC2 * 2 * 128], FP8, kind="ExternalInput")
    selt8 = nc.dram_tensor("selt8", [128, NC_ * 2 * 128], FP8, kind="ExternalInput")
    biasd = nc.dram_tensor("biasd", [128, NC_], F32, kind="ExternalInput")
    outT = nc.dram_tensor("outT", [E, m_core], F16, kind="ExternalOutput")

    xt_v = xt[:, :].rearrange("(c p) m -> p c m", p=128)    # [128, 8, m_core]
    w1_v = w1t[:, :].rearrange("(c p) n -> p c n", p=128)   # [128, 8, 1024]
    w2_v = w2t[:, :].rearrange("(c p) j -> p c j", p=128)   # [128, 8, 1024]

    with tile.TileContext(nc) as tc:
        with (
            tc.tile_pool(name="weights", bufs=1) as wpool,
            tc.tile_pool(name="xt", bufs=n_stripes) as xpool,
            tc.tile_pool(name="u", bufs=16) as upool,
            tc.tile_pool(name="u8", bufs=4) as u8pool,
            tc.tile_pool(name="tmul", bufs=3) as tpool,
            tc.tile_pool(name="at", bufs=8) as apool,
            tc.tile_pool(name="small", bufs=2) as spool,
            tc.tile_pool(name="ostage", bufs=8) as opool,
            tc.tile_pool(name="ps_q", bufs=2, space="PSUM") as psq,
            tc.tile_pool(name="ps_s", bufs=1, space="PSUM") as pss,
            tc.tile_pool(name="ps_rb", bufs=2, space="PSUM") as psrb,
            tc.tile_pool(name="ps_o", bufs=3, space="PSUM") as pso,
        ):
            # Warm the PE's HAM clock gate with small throwaway DR matmuls
            # (same mode as the real stream) while the first DMAs are in
            # flight.  gpsimd memset is ready right after its preamble
            # (~6.5us), before the PE's own preamble ends; N=128 keeps each
            # cold-clock warm matmul cheap so they don't delay stripe 0.
            warm_sb = wpool.tile([128, 2, 128], FP8, name="warm_sb")
            nc.gpsimd.memset(warm_sb[:], 0.0)
            warm_ps = psq.tile([128, MS], F32, tag="q", name="warm_ps")
            for _ in range(14):
                nc.tensor.matmul(
                    warm_ps[:, 0:128], warm_sb[:], warm_sb[:],
                    start=True, stop=True, perf_mode=DR,
                )

            # Weight/selector loads, ordered so stripe 0's mm1 can start
            # as early as possible.
            w1_k = []   # 4 tiles [128, 2, E] fp8 (DoubleRow pairs)
            x_k = [[None] * KC2 for _ in range(n_stripes)]
            # w1 on the sync HWDGE queue, stripe-0 x on the scalar-engine
            # HWDGE queue: both transfer in parallel so stripe 0's first
            # matmul isn't gated by a serial queue (ACT is idle here).
            # Balance stripe 0's 1.5MB of first-needed tiles across the two
            # fast HWDGE issue channels (~768KB each) so mm1 starts earlier
            # (gpsimd's SWDGE path measured too slow to carry weight tiles).
            w1_eng = [nc.sync, nc.sync, nc.scalar, nc.sync]
            x0_eng = [nc.scalar, nc.scalar, nc.scalar, nc.scalar]
            for kc2 in range(KC2):
                t = wpool.tile([128, 2, E], FP8, name=f"w1k{kc2}")
                w1_eng[kc2].dma_start(t[:], w1_v[:, 2 * kc2:2 * kc2 + 2, :])
                w1_k.append(t)
                tx = xpool.tile([128, 2, MS], FP8, tag=f"xt_{kc2}", name=f"xt0_{kc2}")
                x0_eng[kc2].dma_start(tx[:], xt_v[:, 2 * kc2:2 * kc2 + 2, 0:MS])
                x_k[0][kc2] = tx
            sel8_t = wpool.tile([128, KC2, 2, 128], FP8, name="sel8_t")
            nc.sync.dma_start(
                sel8_t[:], sel8[:, :].rearrange("p (c i g) -> p c i g", i=2, g=128)
            )
            selt8_t = wpool.tile([128, NC_, 2, 128], FP8, name="selt8_t")
            nc.sync.dma_start(
                selt8_t[:], selt8[:, :].rearrange("p (c i q) -> p c i q", i=2, q=128)
            )
            bias_t = wpool.tile([128, NC_], F32, name="bias_t")
            nc.sync.dma_start(bias_t[:], biasd[:, :])

            w2_k = []   # 4 tiles [128, 2, E] fp8 (pairs of n-chunks)
            for c2 in range(KC2):
                t = wpool.tile([128, 2, E], FP8, name=f"w2k{c2}")
                nc.sync.dma_start(t[:], w2_v[:, 2 * c2:2 * c2 + 2, :])
                w2_k.append(t)

            def prefetch_x(ms):
                if ms < 1 or ms >= n_stripes:
                    return
                for kc2 in range(KC2):
                    tx = xpool.tile(
                        [128, 2, MS], FP8, tag=f"xt_{kc2}", name=f"xt{ms}_{kc2}"
                    )
                    nc.sync.dma_start(
                        tx[:], xt_v[:, 2 * kc2:2 * kc2 + 2, ms * MS:(ms + 1) * MS]
                    )
                    x_k[ms][kc2] = tx
            prefetch_x(1)

            # rcpc tiles [128, 2, MS] fp8 (K padded to 128 so every matmul
            # in the stream has the same (128,128) tile shape — K=16 tiles
            # measured +285ns/group): [0:16,0,:] = centered reciprocal
            # (DVE-written per stripe), [0,1,:] = A (the decenter constant,
            # written once), rest zero.  Two alternating buffers.
            rcpc_bufs = []
            for i in range(2):
                t = wpool.tile([128, 2, MS], FP8, name=f"rcpc{i}")
                nc.gpsimd.memset(t[:], 0.0)
                nc.gpsimd.memset(t[0:1, 1, :], A_SCALE)
                rcpc_bufs.append(t)

            def emit_rb_at0(prev, ci):
                """rb broadcast DR matmul + the u*rb - A DVE chain -> fp8.
                at0 = u*rb - A is centered (the exact constant A*colsum(W2)
                flows through the f32 bias in the output copy), so mm2
                needs no per-head correction matmul at all."""
                rb_ps = psrb.tile([128, MS], F32, tag="rb", name="rb_ps")
                nc.tensor.matmul(
                    rb_ps[:], selt8_t[:, ci, :, :], prev["rcpc"][:],
                    start=True, stop=True, perf_mode=DR,
                )
                pair, half = divmod(ci, 2)
                t_f = tpool.tile([128, MS], F16, tag="t", name="t_f")
                nc.vector.tensor_mul(t_f[:], prev["u"][ci][:], rb_ps[:])
                nc.vector.tensor_scalar(
                    prev["at"][pair][:, half, :], t_f[:], -A_SCALE, None,
                    op0=ALU.add,
                )

            def emit_mm2(prev, js):
                """j-groups: 4 fp8 DR at0 matmuls each."""
                for j in js:
                    o_ps = pso.tile([128, MS], F32, tag="o", name="o_ps")
                    for c2 in range(KC2):
                        nc.tensor.matmul(
                            o_ps[:],
                            w2_k[c2][:, :, j * 128:(j + 1) * 128],
                            prev["at"][c2][:],
                            start=(c2 == 0),
                            stop=(c2 == KC2 - 1),
                            perf_mode=DR,
                            skip_group_check=True,
                        )
                    o_t = opool.tile([128, MS], F16, tag="ost", name="o_t")
                    nc.scalar.activation(
                        o_t[:], o_ps[:], AF.Identity,
                        bias=bias_t[:, j:j + 1], scale=OUT_DESCALE,
                    )
                    nc.sync.dma_start(
                        outT[j * 128:(j + 1) * 128,
                             prev["ms"] * MS:(prev["ms"] + 1) * MS],
                        o_t[:],
                    )

            def emit_sel(cur):
                """4 DR head-sum matmuls on the fp8 u pairs + rcp chain."""
                s_ps = pss.tile([128, MS], F32, tag="s", name="s_ps")
                for c2 in range(KC2):
                    nc.tensor.matmul(
                        s_ps[:],
                        sel8_t[:, c2, :, :],
                        cur["u8"][c2][:],
                        start=(c2 == 0),
                        stop=(c2 == KC2 - 1),
                        perf_mode=DR,
                    )
                rcp32 = spool.tile([HEADS, MS], F32, tag="rcp32", name="rcp32")
                nc.vector.reciprocal_approx_fast(rcp32[:], s_ps[0:HEADS, :])
                nc.vector.tensor_scalar(
                    cur["rcpc"][0:HEADS, 0, :], rcp32[:], A_SCALE, A_SCALE,
                    op0=ALU.mult, op1=ALU.subtract,
                )

            # Two-deep software pipeline: block i runs mm1(i), the rb/at0
            # chain of stripe i-1, and mm2 of stripe i-2 — so the DVE's
            # at0 chain always has a full block of slack before mm2
            # consumes it, and the tail drain runs fully fed.
            prev = None
            prev2 = None
            for ms in range(n_stripes):
                prefetch_x(ms + 1)
                cur = {
                    "ms": ms,
                    "u": [],
                    "u8": [
                        u8pool.tile([128, 2, MS], FP8, tag=f"u8{p}", name=f"u8{ms}_{p}")
                        for p in range(KC2)
                    ],
                    "at": [
                        apool.tile([128, 2, MS], FP8, tag=f"at{p}", name=f"at{ms}_{p}")
                        for p in range(KC2)
                    ],
                    "rcpc": rcpc_bufs[ms % 2],
                }

                # Head of block: 2 rb matmuls of the previous stripe; the
                # rest interleave into the mm1 loop so the PE stays just
                # ahead of the DVE's 823ns/op at0 chain (psrb has 2 bufs).
                if prev is not None:
                    emit_rb_at0(prev, 0)
                    emit_rb_at0(prev, 1)

                for ci in range(NC_):
                    q_ps = psq.tile([128, MS], F32, tag="q", name="q_ps")
                    for kc2 in range(KC2):
                        nc.tensor.matmul(
                            q_ps[:],
                            w1_k[kc2][:, :, ci * 128:(ci + 1) * 128],
                            x_k[ms][kc2][:],
                            start=(kc2 == 0),
                            stop=(kc2 == KC2 - 1),
                            perf_mode=DR,
                        )
                    u_t = upool.tile([128, MS], F16, tag="u", name="u_t")
                    nc.scalar.activation(u_t[:], q_ps[:], AF.Exp, scale=1.0 / 128.0)
                    cur["u"].append(u_t)
                    if prev is not None and ci + 2 < NC_:
                        emit_rb_at0(prev, ci + 2)

                # u8 copies (DVE) emitted after all at0 ops so the at0
                # chain (needed first, by mm2) drains first.
                for ci in range(NC_):
                    nc.vector.tensor_copy(
                        cur["u8"][ci // 2][:, ci % 2, :], cur["u"][ci][:]
                    )

                if prev2 is not None:
                    emit_mm2(prev2, range(6))
                emit_sel(cur)
                if prev2 is not None:
                    emit_mm2(prev2, range(6, NC_))
                prev2, prev = prev, cur

            # drain block: last stripe's rb/at0 chain interleaved with the
            # second-to-last stripe's output projection
            emit_rb_at0(prev, 0)
            emit_rb_at0(prev, 1)
            for k in range(6):
                if prev2 is not None:
                    emit_mm2(prev2, [k])
                emit_rb_at0(prev, k + 2)
            if prev2 is not None:
                emit_mm2(prev2, range(6, NC_))
            # epilogue: last stripe's output projection, fully fed
            emit_mm2(prev, range(NC_))
    nc.compile()
    return nc


_NC_CACHE = None
LAST_RESULT = None


def _ensure_ntff_hook():
    """bass_utils' axon trace path needs antenv.axon_hooks, which this
    container's antenv lacks. Provide it + register the ctypes NTFF hook."""
    import types

    try:
        from antenv.axon_hooks import get_axon_ntff_profile_hook  # noqa: F401
        return True
    except ImportError:
        pass
    try:
        import antenv
        from trn_agent_boot.trn_boot import _ntff_profile_via_ctypes

        m = types.ModuleType("antenv.axon_hooks")
        state = {"hook": None}
        m.set_axon_ntff_profile_hook = lambda h: state.__setitem__("hook", h)
        m.get_axon_ntff_profile_hook = lambda: state["hook"]
        sys.modules["antenv.axon_hooks"] = m
        antenv.axon_hooks = m
        m.set_axon_ntff_profile_hook(
            _ntff_profile_via_ctypes("/opt/axon/libaxon_pjrt.so")
        )
        return True
    except Exception as e:  # pragma: no cover
        print(f"ntff hook injection failed: {e}")
        return False


def _selectors():
    # head index of global feature n is n // 64; chunk ci covers n in
    # [128ci, 128ci+128) -> heads 2ci (partitions 0..63), 2ci+1 (64..127)
    # All selector tiles are padded to the uniform (128, 2, 128) shape.
    sel8 = np.zeros((128, KC2, 2, 128), np.float32)
    for c2 in range(KC2):
        for i in range(2):
            ci = 2 * c2 + i
            sel8[:64, c2, i, 2 * ci] = 1.0 / 64.0
            sel8[64:, c2, i, 2 * ci + 1] = 1.0 / 64.0
    # selt8[:, ci, 0, :]: 0/1 head selector; [:, ci, 1, :]: row 0 = 1.0,
    # which multiplies the constant-A row of rcpc_pad -> rb = rcpc + A.
    selt8 = np.zeros((128, NC_, 2, 128), np.float32)
    for ci in range(NC_):
        selt8[2 * ci, ci, 0, :64] = 1.0
        selt8[2 * ci + 1, ci, 0, 64:] = 1.0
        selt8[0, ci, 1, :] = 1.0
    return (
        np.ascontiguousarray(sel8.reshape(128, KC2 * 2 * 128)).astype(_E4),
        np.ascontiguousarray(selt8.reshape(128, NC_ * 2 * 128)).astype(_E4),
    )


def _prep_weights(W1, W2):
    w1t = np.ascontiguousarray(W1[:E, :].T * W_SCALE).astype(_E4)  # [k, n]
    w2t = np.ascontiguousarray(W2.T * W_SCALE).astype(_E4)         # [n, j]
    # exact column sums of 32*W2T (fp32, TRUE weights — this is what keeps
    # the W2 fp8 error coupled only to the centered at0)
    colsum = (W2.T * W_SCALE).sum(axis=0)
    bias = (colsum * A_SCALE * OUT_DESCALE).astype(np.float32)
    biasd = np.ascontiguousarray(bias.reshape(NC_, 128).T)         # [128, NC_]
    return w1t, w2t, biasd


def kernel(x, W1, W2, heads, trace=False):
    global _NC_CACHE, LAST_RESULT
    x = np.asarray(x, dtype=np.float32)
    W1 = np.asarray(W1, dtype=np.float32)
    W2 = np.asarray(W2, dtype=np.float32)

    X = x.reshape(M_TOTAL, E)
    XT8 = np.ascontiguousarray(X.T).astype(_E4)  # [E, M_TOTAL]
    w1t, w2t, biasd = _prep_weights(W1, W2)
    sel8, selt8 = _selectors()

    in_maps = []
    for c in range(N_CORES):
        xt_c = np.ascontiguousarray(XT8[:, c * M_CORE:(c + 1) * M_CORE])
        in_maps.append(
            {"xt": xt_c, "w1t": w1t, "w2t": w2t, "sel8": sel8,
             "selt8": selt8, "biasd": biasd}
        )

    if _NC_CACHE is None:
        _NC_CACHE = build_nc()

    if trace:
        trace = _ensure_ntff_hook()

    res = run_bass_kernel_spmd(_NC_CACHE, in_maps, list(range(N_CORES)), trace=trace)
    LAST_RESULT = res

    OT = np.concatenate(
        [np.asarray(res.results[c]["outT"]).astype(np.float32) for c in range(N_CORES)],
        axis=1,
    )
    return np.ascontiguousarray(OT.T).reshape(B, S, E)
